# revision 1
# baseline (speedup 1.0000x reference)
"""Trainium2 Bass kernel for nn_AttentionLayer (B=4, N=4096, D=128).

Computation (per reference):
    Q = h @ Wq + bq ; K = h @ Wk + bk ; V = h @ Wv + bv          [B, N, 128]
    scores = einsum("bnd,bmd->bnm", K, Q) / sqrt(128)            [B, N, N]
    attn = softmax(scores, axis=-1)
    out = einsum("bnm,bmd->bnd", attn, V)                        [B, N, N->128]

Sharding: 8 cores = 4 batches x 2 chunks of 2048 K-rows (output rows).
Fully data-parallel SPMD - no collectives. Each core receives its batch's
h rows PERMUTED so that its own K-chunk rows come first: softmax/PV reduce
over the m (Q/V) index, which is order-independent, so the permutation only
fixes which rows the core treats as its K chunk (the first 2048).

Per-core kernel (all on-chip after one 2MB load):
  hT   = h^T (PE transposes)                      [128c, 4096m]
  kT   = Wk^T-proj of hT[:, :2048] + bk           [128d, 2048n]
  qT   = Wq^T-proj of hT + bq                     [128d, 4096m]
  vT -> V natural tiles (PE transposes)           [4096m, 128d]
  per (nh half of n, mi of 32 m-tiles):
      sT  = qT_mi^T @ kT_nh            (PSUM, fp32r)     [128m, 1024n]
      eT  = exp(sT / sqrt(128))        (ACT, 2x512)      [128m, 1024n]
      oT += V_mi^T @ eT                (PSUM accum)      [128d, 1024n]
      den += ones^T @ eT               (PSUM accum)      [128d(bcast), 1024n]
  out_nh = (oT * 1/den)^T              (DVE + PE transposes) -> DMA out
"""

import math
from contextlib import ExitStack

import numpy as np

import concourse.bass as bass
import concourse.mybir as mybir
import concourse.tile as tile
from concourse.bass_utils import run_bass_kernel_spmd
from concourse.masks import make_identity
from concourse.tile import ScopedClock

F32 = mybir.dt.float32
F32R = mybir.dt.float32r
BF16 = mybir.dt.bfloat16

B, N, D = 4, 4096, 128
NCORES = 8
CHUNK = N * B // NCORES  # 2048 output rows per core
NW = 1024  # n processed per PSUM-resident accumulation group
SCALE = 1.0 / math.sqrt(D)


def _patched_drain_and_barrier(self, tick_clock, wait_clock):
    # This walrus build rejects multiple sync waits on the Drain CTRL
    # instruction. Carry the waits on preceding SP nops (same engine =>
    # program order) and leave the drain nearly bare.
    nc = self.nc
    carrier = nc.sync.nop(nofuse=True, hint="drain_waits")
    wait_clock.add_sem_waits(carrier.ins, ScopedClock({None: tick_clock.global_clock}))
    si = carrier.ins.sync_info
    waits = list(si.on_wait) if si is not None else []
    if len(waits) > 1:
        by_name = {}
        for h in self.sems.allocated().values():
            by_name[getattr(h, "name", None) or str(h)] = h
        si.on_wait = [waits[0]]
        for w in waits[1:]:
            n = nc.sync.nop(nofuse=True, hint="drain_waits2")
            n.wait_op(by_name[w.ant_name], w.wait_value, "sem-ge")
    nc.sync.drain()
    nc.all_engine_barrier()
    assert self.sems is not None
    popped = nc._tile_sem_poison_stack.pop()
    assert popped is self._sem_poison
    nc.clear_and_free_semaphores(list(self.sems.allocated().values()))
    nc.all_engine_barrier()


def ts(i, sz):
    return slice(i * sz, (i + 1) * sz)


def _split_excess_waits(nc, maxw=1):
    # This walrus build allows at most ~1 sync wait per lowered instruction.
    # Hoist excess waits onto preceding same-engine NoOps.
    cnt = 0
    for f in nc.m.functions:
        for bb in f.blocks:
            out = []
            for inst in bb.instructions:
                si = inst.sync_info
                waits = list(si.on_wait) if si is not None else []
                if len(waits) > maxw:
                    for w in waits[: len(waits) - maxw]:
                        nop = mybir.InstNoOp(
                            name=f"{inst.name}-hw{cnt}",
                            engine=inst.engine,
                            ins=[],
                            outs=[],
                            sync_info=mybir.SyncInfo(on_wait=[w], on_update=[]),
                        )
                        out.append(nop)
                        cnt += 1
                    si.on_wait = waits[len(waits) - maxw :]
                out.append(inst)
            bb.instructions = out
    return cnt


def build_nc(n=N, chunk=CHUNK, nw=NW, split_waits=True, repeat=1, mm_dtype="f32r"):
    MMD = BF16 if mm_dtype == "bf16" else F32R
    M_TILES = n // 128
    NH = chunk // nw
    MMW = min(512, nw)  # matmul moving width (PSUM bank cap for fp32)
    GRP = min(nw // 128, M_TILES)  # transposes batched per psum tile
    GW = GRP * 128  # group width in columns
    DMA_GRP = min(4, M_TILES)  # h tiles per input DMA
    tile.TileContext._drain_and_barrier = _patched_drain_and_barrier
    nc = bass.Bass("TRN2", target_bir_lowering=False, debug=False, num_devices=NCORES)

    h_d = nc.dram_tensor("h", [n, D], F32, kind="ExternalInput")
    w_d = nc.dram_tensor("wqkv", [3, D, D], F32, kind="ExternalInput")
    b_d = nc.dram_tensor("bqkv", [3, D], F32, kind="ExternalInput")
    out_d = nc.dram_tensor("out", [chunk, D], F32, kind="ExternalOutput")

    with tile.TileContext(nc) as tc, ExitStack() as ctx:
        consts = ctx.enter_context(tc.tile_pool(name="consts", bufs=1))
        big = ctx.enter_context(tc.tile_pool(name="big", bufs=1))
        stage = ctx.enter_context(tc.tile_pool(name="stage", bufs=1))
        expp = ctx.enter_context(tc.tile_pool(name="expp", bufs=3))
        denp = ctx.enter_context(tc.tile_pool(name="denp", bufs=2))
        outn = ctx.enter_context(tc.tile_pool(name="outn", bufs=2))
        outsp = ctx.enter_context(tc.tile_pool(name="outs", bufs=2))
        s_bufs = 4 if nw <= 512 else 2
        od_bufs = 2 if nw <= 512 else 1
        ps_s = ctx.enter_context(tc.tile_pool(name="ps_s", bufs=s_bufs, space="PSUM"))
        ps_o = ctx.enter_context(tc.tile_pool(name="ps_o", bufs=od_bufs, space="PSUM"))
        ps_d = ctx.enter_context(tc.tile_pool(name="ps_d", bufs=od_bufs, space="PSUM"))

        # ---- constants (2 DMAs + DVE casts) ----
        w_s = consts.tile([D, 3, D], F32, tag="w_s")
        b_s = consts.tile([D, 3], F32, tag="b_s")
        nc.sync.dma_start(out=w_s, in_=w_d.ap().rearrange("w c d -> c w d"))
        nc.sync.dma_start(out=b_s, in_=b_d.ap().rearrange("w d -> d w"))
        wq_r = consts.tile([D, D], MMD, tag="wq_r")
        wk_r = consts.tile([D, D], MMD, tag="wk_r")
        wv_r = consts.tile([D, D], MMD, tag="wv_r")
        nc.vector.tensor_copy(out=wq_r, in_=w_s[:, 0, :])
        nc.vector.tensor_copy(out=wk_r, in_=w_s[:, 1, :])
        nc.vector.tensor_copy(out=wv_r, in_=w_s[:, 2, :])
        bq_s, bk_s, bv_s = b_s[:, 0:1], b_s[:, 1:2], b_s[:, 2:3]
        ident = consts.tile([128, 128], F32, tag="ident")
        ones_f = consts.tile([128, 128], F32, tag="ones_f")
        ones = consts.tile([128, 128], MMD, tag="ones")
        make_identity(nc, ident)
        nc.vector.memset(ones_f, 1.0)
        nc.vector.tensor_copy(out=ones, in_=ones_f)

        # ---- load h ----
        h_r = h_d.ap().rearrange("(g t p) c -> p g t c", p=128, t=DMA_GRP)
        h_stage = stage.tile([128, M_TILES // DMA_GRP, DMA_GRP, 128], F32, tag="h_st")
        for i in range(M_TILES // DMA_GRP):
            nc.sync.dma_start(out=h_stage[:, i, :, :], in_=h_r[:, i, :, :])
        h_flat = h_stage.rearrange("p g t c -> p (g t) c")

        def body():
            # ---- hT via PE transposes ----
            hT = big.tile([128, n], MMD, tag="hT")
            for g in range(M_TILES // GRP):
                t_ps = ps_s.tile([128, GW], F32, tag="s")
                for k in range(GRP):
                    i = g * GRP + k
                    nc.tensor.transpose(t_ps[:, ts(k, 128)], h_flat[:, i, :], ident)
                nc.vector.tensor_copy(out=hT[:, ts(g, GW)], in_=t_ps[:, :GW])

            # ---- projections (fp32r matmuls; bias via DVE tensor_scalar) ----
            # kT first: the main loop needs all of kT but only qT group 0.
            qT = big.tile([128, n], MMD, tag="qT")
            kT = big.tile([128, chunk], MMD, tag="kT")
            vT = big.tile([128, n], F32, tag="vT")
            for dst, w_r, bias, width in (
                (kT, wk_r, bk_s, chunk),
                (qT, wq_r, bq_s, n),
                (vT, wv_r, bv_s, n),
            ):
                gw = min(nw, width)
                for g in range(width // gw):
                    p_t = ps_s.tile([128, nw], F32, tag="s")
                    for j in range(gw // MMW):
                        nc.tensor.matmul(
                            p_t[:, ts(j, MMW)],
                            w_r,
                            hT[:, g * gw + j * MMW : g * gw + (j + 1) * MMW],
                        )
                    nc.vector.tensor_scalar_add(
                        out=dst[:, ts(g, gw)], in0=p_t[:, :gw], scalar1=bias
                    )

            # ---- V natural tiles (transpose vT) ----
            vN = big.tile([128, n], MMD, tag="vN")  # tile i cols [128i,128i+128)
            for g in range(M_TILES // GRP):
                t_ps = ps_s.tile([128, GW], F32, tag="s")
                for k in range(GRP):
                    i = g * GRP + k
                    nc.tensor.transpose(t_ps[:, ts(k, 128)], vT[:, ts(i, 128)], ident)
                nc.vector.tensor_copy(out=vN[:, ts(g, GW)], in_=t_ps[:, :GW])

            # ---- main attention loop ----
            out_r = out_d.ap().rearrange("(t p) d -> p t d", p=128)
            for nh in range(NH):
                o_t = ps_o.tile([128, nw], F32, tag="o")
                d_t = ps_d.tile([128, nw], F32, tag="d")

                def emit_scores(mi):
                    s_t = ps_s.tile([128, nw], F32, tag="s")
                    for j in range(nw // MMW):
                        nc.tensor.matmul(
                            s_t[:, ts(j, MMW)],
                            qT[:, ts(mi, 128)],
                            kT[:, nh * nw + j * MMW : nh * nw + (j + 1) * MMW],
                        )
                    return s_t

                # software pipeline: keep scores(mi+1) AHEAD of pv/den(mi) in
                # the static PE program so the PE never idles waiting for exp.
                s_next = emit_scores(0)
                for mi in range(M_TILES):
                    s_t = s_next
                    e_t = expp.tile([128, nw], MMD, tag="e")
                    nc.scalar.activation(
                        out=e_t,
                        in_=s_t,
                        func=mybir.ActivationFunctionType.Exp,
                        scale=SCALE,
                    )
                    if mi + 1 < M_TILES:
                        s_next = emit_scores(mi + 1)
                    first, last = mi == 0, mi == M_TILES - 1
                    for j in range(nw // MMW):
                        nc.tensor.matmul(
                            o_t[:, ts(j, MMW)],
                            vN[:, ts(mi, 128)],
                            e_t[:, ts(j, MMW)],
                            start=first,
                            stop=last,
                            skip_group_check=True,
                        )
                    for j in range(nw // MMW):
                        nc.tensor.matmul(
                            d_t[:, ts(j, MMW)],
                            ones,
                            e_t[:, ts(j, MMW)],
                            start=first,
                            stop=last,
                            skip_group_check=True,
                        )
                rden = denp.tile([128, nw], F32, tag="rden")
                nc.vector.reciprocal(out=rden, in_=d_t)
                o_n = outn.tile([128, nw], F32, tag="o_n")
                nc.vector.tensor_mul(out=o_n, in0=o_t, in1=rden)
                t_ps = ps_s.tile([128, nw], F32, tag="s")
                for k in range(nw // 128):
                    nc.tensor.transpose(t_ps[:, ts(k, 128)], o_n[:, ts(k, 128)], ident)
                o_s = outsp.tile([128, nw // 128, 128], F32, tag="o_s")
                nc.vector.tensor_copy(
                    out=o_s, in_=t_ps.rearrange("p (t d) -> p t d", d=128)
                )
                nc.sync.dma_start(
                    out=out_r[:, nh * (nw // 128) : (nh + 1) * (nw // 128), :], in_=o_s
                )

        if repeat > 1:
            with tc.For_i(0, repeat, 1):
                body()
        else:
            body()

    if split_waits:
        _split_excess_waits(nc)
    return nc


_NC_CACHE = None
_LAST_RESULTS = None
TRACE = False
REPEAT = 1


def kernel(h_a, Wq, bq, Wk, bk, Wv, bv):
    global _NC_CACHE, _LAST_RESULTS
    h_a = np.ascontiguousarray(h_a, dtype=np.float32)
    if _NC_CACHE is None:
        _NC_CACHE = build_nc(repeat=REPEAT)
    nc = _NC_CACHE

    consts = {
        "wqkv": np.ascontiguousarray(np.stack([Wq, Wk, Wv]), np.float32),
        "bqkv": np.ascontiguousarray(np.stack([bq, bk, bv]), np.float32),
    }
    in_maps = []
    for core in range(NCORES):
        b, half = divmod(core, 2)
        n0 = half * CHUNK
        # chunk rows first, the rest after (order of the tail is irrelevant)
        perm = np.concatenate(
            [h_a[b, n0 : n0 + CHUNK], h_a[b, : n0], h_a[b, n0 + CHUNK :]], axis=0
        )
        in_maps.append({"h": np.ascontiguousarray(perm), **consts})

    res = run_bass_kernel_spmd(
        nc, in_maps, core_ids=list(range(NCORES)), trace=TRACE
    )
    _LAST_RESULTS = res

    out = np.empty((B, N, D), np.float32)
    for core in range(NCORES):
        b, half = divmod(core, 2)
        n0 = half * CHUNK
        out[b, n0 : n0 + CHUNK] = res.results[core]["out"]
    return out

